# revision 3
# baseline (speedup 1.0000x reference)
"""Trainium2 Bass kernel for BinaryTimedPSP (causal boxcar window sum + clip).

psp[t] = clip(sum_{k=max(0,t-D+1)}^{t} x[k], 0, 1) along time axis of a
[T=2048, B=16, N=2048] f32 spike tensor, D = duration (100).

Strategy: pure data-parallel over the 8 NeuronCores — the flattened B*N axis
(32768 columns) is split into 8 slabs of 4096 columns. Each core processes a
[T, 4096] slab:
  - time is tiled into 16 chunks of 128 rows, loaded as [128 part, 4096 free]
  - the window sum of chunk i is a block-banded matmul:
      out_i = A_0 @ x_i + A_1 @ x_{i-1} (+ ... for D > 128)
    where A_m[r, c] = 1 iff 0 <= (r + 128*m) - c < D. Exact for 0/1 spikes.
  - matmuls run as float32r (1 cycle/row at N=512) accumulating in f32 PSUM
  - clip to [0,1] == min(., 1.0) since the sum is >= 0, fused into the
    PSUM->SBUF copy on the vector engine
No cross-core communication; the gather is a host-side concatenate.
"""

import numpy as np

T_FULL, B_FULL, N_FULL = 2048, 16, 2048
NCORES = 8
P = 128
COLS = B_FULL * N_FULL          # 32768
FREE = COLS // NCORES           # 4096 columns per core
NCHUNK = T_FULL // P            # 16 time chunks
FTILE = 512                     # one PSUM bank of f32
NFT = FREE // FTILE             # 8

_CACHE: dict = {}


def _n_mats(d: int) -> int:
    # number of 128x128 band blocks: block m covers lags [128m-127, 128m+127]
    n = (d + P - 2) // P + 1 if d > 1 else 1
    n = max(1, min(n, NCHUNK))
    # exact condition: include m while 128m - 127 <= d - 1
    n = 1
    while P * n - (P - 1) <= d - 1 and n < NCHUNK:
        n += 1
    return n


def _weights(d: int, n_mats: int) -> np.ndarray:
    # W[m*128 + c, r] = A_m[r, c] = 1 iff 0 <= (r + 128m) - c < d
    # (lhsT layout: partition dim = contraction c, free dim = output row r)
    r = np.arange(P)[None, :]
    c = np.arange(P)[:, None]
    mats = []
    for m in range(n_mats):
        diff = r + P * m - c
        mats.append(((diff >= 0) & (diff < d)).astype(np.float32))
    return np.concatenate(mats, axis=0)


def _build(d: int):
    import concourse.bacc as bacc
    import concourse.mybir as mybir
    from concourse.tile import TileContext

    n_mats = _n_mats(d)
    f32 = mybir.dt.float32
    f32r = mybir.dt.float32r

    nc = bacc.Bacc(None)
    x = nc.dram_tensor("x", [T_FULL, FREE], f32r, kind="ExternalInput")
    w = nc.dram_tensor("w", [n_mats * P, P], f32r, kind="ExternalInput")
    y = nc.dram_tensor("y", [T_FULL, FREE], f32, kind="ExternalOutput")
    xr = x.rearrange("(n p) f -> n p f", p=P)
    yr = y.rearrange("(n p) f -> n p f", p=P)
    wr = w.rearrange("(m p) q -> m p q", p=P)

    with TileContext(nc) as tc:
        with (
            tc.tile_pool(name="wpool", bufs=1) as wpool,
            tc.tile_pool(name="xpool", bufs=4) as xpool,
            tc.tile_pool(name="opool", bufs=3) as opool,
            tc.tile_pool(name="ppool", bufs=8, space="PSUM") as ppool,
        ):
            wts = []
            for m in range(n_mats):
                wt = wpool.tile([P, P], f32r, tag=f"w{m}")
                nc.sync.dma_start(out=wt, in_=wr[m])
                wts.append(wt)
            xs = []
            for i in range(NCHUNK):
                xt = xpool.tile([P, FREE], f32r)
                nc.sync.dma_start(out=xt, in_=xr[i])
                xs.append(xt)
                ot = opool.tile([P, FREE], f32)
                terms = [m for m in range(n_mats) if i - m >= 0]
                for f in range(NFT):
                    ps = ppool.tile([P, FTILE], f32)
                    fs = slice(f * FTILE, (f + 1) * FTILE)
                    for j, m in enumerate(terms):
                        nc.tensor.matmul(
                            ps,
                            wts[m],
                            xs[i - m][:, fs],
                            start=(j == 0),
                            stop=(j == len(terms) - 1),
                        )
                    nc.vector.tensor_scalar_min(out=ot[:, fs], in0=ps, scalar1=1.0)
                nc.sync.dma_start(out=yr[i], in_=ot)
    nc.finalize()
    return nc, n_mats


def _get_built(d: int):
    if d not in _CACHE:
        _CACHE[d] = _build(d)
    return _CACHE[d]


def kernel(input_spikes, duration, _trace=False):
    from concourse.bass_utils import run_bass_kernel_spmd

    x = np.ascontiguousarray(np.asarray(input_spikes, dtype=np.float32))
    d = int(duration)
    assert x.shape == (T_FULL, B_FULL, N_FULL), x.shape

    nc, n_mats = _get_built(d)
    W = _weights(d, n_mats)

    xf = x.reshape(T_FULL, COLS)
    in_maps = [
        {"x": np.ascontiguousarray(xf[:, c * FREE : (c + 1) * FREE]), "w": W}
        for c in range(NCORES)
    ]
    res = run_bass_kernel_spmd(
        nc, in_maps, core_ids=list(range(NCORES)), trace=_trace
    )
    out = np.concatenate([r["y"] for r in res.results], axis=1)
    out = out.reshape(T_FULL, B_FULL, N_FULL)
    if _trace:
        return out, res
    return out
